# revision 23
# baseline (speedup 1.0000x reference)
"""Trainium2 Bass kernel for nn_COGV1 (foveated multi-scale conv + BN + pool + conv).

v2: 4-image-merged staging DMAs, PE row-tiling (32x128 quadrants) for conv1,
pool-before-BN reorder (raw expansion, -inf pad ring, BN+ReLU fused after
pool), BN stats fused into conv1 via tensor_tensor_reduce, cost-model-driven
engine placement for expansion copies.
"""
import numpy as np

try:
    import concourse.bass as bass  # noqa: F401
except ImportError:  # grading env may not have the repo on sys.path yet
    import sys
    for _p in ('/opt/trn_rl_repo', '/root/.axon_site/_ro/trn_rl_repo'):
        if _p not in sys.path:
            sys.path.insert(0, _p)
    import concourse.bass as bass  # noqa: F401

import concourse.bacc as bacc
import concourse.tile as tile
from concourse import mybir
from concourse.bass import AP
from concourse.bass_utils import run_bass_kernel_spmd

F32 = mybir.dt.float32
BF16 = mybir.dt.bfloat16

OC = 128; KS = 7; STRIDE = 2; IMG = 224; NS = 7; PAD = 3; EPS = 1e-5
OUT_HW = 112
XPAD = 230   # padded image size
PHW = 115    # phase image width
N_CORES = 8
NB = 4       # images per core
NPIX1 = 32 * OUT_HW * OUT_HW   # BN1 count (full batch)
NPIX2 = 32 * 56 * 56           # BN2 count
CV = 113     # canvas with 1-px -inf pad at top/left
NEG = -1.0e30

# DRAM phase layout [row 230][c 3][u 115][n 4] (n innermost for long DMA lines)
S_U = NB                  # 4
S_C = PHW * NB            # 460
S_ROW = 3 * S_C           # 1380
REP_SZ = XPAD * S_ROW


class _Win:
    pass


class _StopBuild(Exception):
    pass


def _decompose(idx, n_out, n_in):
    """pieces (o0, ostep, i0, istep, cnt): out[o0+k*ostep] = in[i0+k*istep]."""
    idx = np.asarray(idx)
    runs = []
    j = 0
    while j < n_out:
        j0 = j
        while j + 1 < n_out and idx[j + 1] == idx[j] + 1:
            j += 1
        runs.append((j0, 1, int(idx[j0]), 1, j - j0 + 1))
        j += 1
    g = int(np.gcd(n_out, n_in))
    p_out, p_in = n_out // g, n_in // g
    per = [(u, p_out, int(idx[u]), p_in, g) for u in range(p_out)]
    ok = all(idx[u + k * p_out] == idx[u] + k * p_in
             for u in range(p_out) for k in range(g))
    if ok and len(per) < len(runs):
        return per
    return runs


def _windows():
    scales = np.linspace(2.0, 1.0, NS, dtype=np.float32)
    borders = np.linspace(0, IMG // 2, NS + 1).astype(np.int64)
    iw = np.empty((NS, 4, 4), dtype=np.int64)
    for s in range(NS):
        a, b = borders[s], borders[s + 1]
        c, d = IMG - b, IMG - a
        iw[s] = np.array([[a, a, b, c], [b, a, d, b], [c, b, d, d], [a, c, c, d]])
    ow = iw // STRIDE
    iwp = iw + PAD * 2
    iwp = np.stack([(iwp[s].astype(np.float32) / scales[s]).astype(np.int64)
                    for s in range(NS)])
    iwp[:, :, :2] -= PAD
    iwp[:, :, 2:] += PAD
    wins = []
    foff = 0
    for s in range(NS):
        for w in range(4):
            W = _Win()
            W.ti, W.li, W.bi, W.ri = (int(v) for v in iwp[s, w])
            W.fh = (W.bi - W.ti - KS) // STRIDE + 1
            W.fw = (W.ri - W.li - KS) // STRIDE + 1
            W.to, W.lo, W.bo, W.ro = (int(v) for v in ow[s, w])
            W.oh, W.ow = W.bo - W.to, W.ro - W.lo
            W.ih = (np.arange(W.oh) * W.fh) // W.oh
            W.iw = (np.arange(W.ow) * W.fw) // W.ow
            W.wrow = np.bincount(W.ih, minlength=W.fh)
            W.wcol = np.bincount(W.iw, minlength=W.fw)
            W.rp = _decompose(W.ih, W.oh, W.fh)
            W.cp = _decompose(W.iw, W.ow, W.fw)
            W.row_id = (W.oh == W.fh and len(W.rp) == 1
                        and tuple(W.rp[0]) == (0, 1, 0, 1, W.oh))
            W.foff = foff; foff += W.fh * W.fw + (W.fh * W.fw) % 2
            wins.append(W)
    return wins, foff


def _sub_ap(tile_ap, extra_off, dims):
    """AP over a tile's tensor: keep partition dim, custom free dims."""
    return AP(tensor=tile_ap.tensor, offset=tile_ap.offset + extra_off,
              ap=[list(tile_ap.ap[0])] + [list(d) for d in dims])


def _dram_ap(handle, off, dims):
    return AP(tensor=handle, offset=off, ap=[list(d) for d in dims])


def _build(conv_w, gamma1, beta1, conv2_w, gamma2, beta2,
           stop_after=None, no_cc=False, use_tiling=True,
           no_exp=False, no_stats=False):
    import ml_dtypes
    import contextlib
    bf = lambda x: np.asarray(x, ml_dtypes.bfloat16)
    wins, NFC = _windows()

    nc = bacc.Bacc("TRN2", target_bir_lowering=False, debug=False,
                   num_devices=N_CORES)
    inp_d = nc.declare_dram_parameter("inp", [NB, 3, IMG, IMG], F32, isOutput=False)
    out_d = nc.declare_dram_parameter("out", [NB, OC, 56, 56], BF16, isOutput=True)

    # conv1 weights: rows k = ky*3 + c; taps 0..3 = kx 0,2,4,6; 4..6 = kx 1,3,5
    # replicated into 4 partition quadrants for PE row tiling.
    l1 = np.zeros((128, 7, OC), np.float32)
    for ky in range(KS):
        for c in range(3):
            for q in range(4):
                for t in range(4):
                    l1[32 * q + ky * 3 + c, t] = conv_w[:, c, ky, 2 * t]
                for t in range(3):
                    l1[32 * q + ky * 3 + c, 4 + t] = conv_w[:, c, ky, 2 * t + 1]
    l1_d = nc.inline_tensor(bf(l1), name="l1")
    w2 = np.transpose(conv2_w, (1, 2, 3, 0)).reshape(OC, 9, OC)
    w2_d = nc.inline_tensor(bf(w2), name="w2")
    wd = np.zeros(NFC, np.float32)
    for W in wins:
        wd[W.foff:W.foff + W.fh * W.fw] = np.outer(W.wrow, W.wcol).reshape(-1)
    wd_d = nc.inline_tensor(bf(wd), name="wd")
    gb1_d = nc.inline_tensor(np.stack([gamma1, beta1], 1).astype(np.float32), name="gb1")
    gb2_d = nc.inline_tensor(np.stack([gamma2, beta2], 1).astype(np.float32), name="gb2")

    xe_d = nc.dram_tensor("xe_rep", [REP_SZ + 64], BF16)
    xo_d = nc.dram_tensor("xo_rep", [REP_SZ + 64], BF16)

    # running engine load (ns) for copy placement; pre-biased for known
    # conv1-phase work issued outside the greedy (stats+copies on v, etc).
    ecost = {'v': 0.0, 'a': 0.0, 'g': 0.0, 'ds': 0.0}

    with tile.TileContext(nc) as tc:
        ctx = contextlib.ExitStack()
        try:
          with ctx:
            consts_p = ctx.enter_context(tc.tile_pool(name="consts", bufs=1))
            stage_p = ctx.enter_context(tc.tile_pool(name="stage", bufs=3))
            scr_p = ctx.enter_context(tc.tile_pool(name="scr", bufs=1))
            psum_p = ctx.enter_context(tc.tile_pool(name="psum", bufs=8, space="PSUM"))
            big_p = ctx.enter_context(tc.tile_pool(name="big", bufs=1))
            rw_p = ctx.enter_context(tc.tile_pool(name="rwp", bufs=2))
            small_p = ctx.enter_context(tc.tile_pool(name="small", bufs=1))
            dram_p = ctx.enter_context(tc.tile_pool(name="dramp", bufs=1, space="DRAM"))

            # ---- constants ----
            w1sb = consts_p.tile([128, 7, OC], BF16)
            w2sb = consts_p.tile([OC, 9, OC], BF16)
            gb1 = consts_p.tile([OC, 2], F32)
            gb2 = consts_p.tile([OC, 2], F32)
            # wdsb shares the (later) c2raw slot: disjoint live ranges
            wdsb = big_p.tile([OC, NFC], BF16, tag="c2raw", name="wdsb")
            nc.scalar.dma_start(out=w1sb, in_=l1_d[:, :, :])
            nc.scalar.dma_start(out=w2sb, in_=w2_d[:, :, :])
            nc.scalar.dma_start(out=wdsb, in_=_dram_ap(wd_d, 0, [[0, OC], [1, NFC]]))
            nc.scalar.dma_start(out=gb1, in_=gb1_d[:, :])
            nc.scalar.dma_start(out=gb2, in_=gb2_d[:, :])

            # ---- phase split: [row][c][n][u] bf16 replicas in DRAM ----
            for half in range(2):
                r0 = half * PHW
                xe_t = stage_p.tile([PHW, 3, PHW, NB], BF16, tag="so",
                                    name=f"xe_t{half}")
                xo_t = stage_p.tile([PHW, 3, PHW, NB], BF16, tag="so",
                                    name=f"xo_t{half}")
                nc.vector.memset(xe_t[:, :, :, :], 0.0)
                nc.vector.memset(xo_t[:, :, :, :], 0.0)
                for n in range(NB):
                    in_t = stage_p.tile([128, 3, IMG], F32, tag="se",
                                        name=f"in_t{half}_{n}")
                    lo = max(0, r0 - PAD)
                    hi = min(IMG, r0 + PHW - PAD)
                    p_lo = lo - (r0 - PAD)
                    if p_lo > 0:
                        nc.vector.memset(in_t[0:p_lo, :, :], 0.0)
                    vr = p_lo + hi - lo   # valid rows; rows beyond stay 0 in xe/xo
                    (nc.sync if n % 2 == 0 else nc.scalar).dma_start(
                        out=in_t[p_lo:p_lo + hi - lo, :, :],
                        in_=inp_d[n, :, lo:hi, :].transpose([1, 0, 2]))
                    ecost['ds' if n % 2 == 0 else 'a'] += 1800
                    # xe[u] = xpad[2u]   = inp[2u-3] -> cols 1,3,..,223 -> u 2..113
                    # xo[u] = xpad[2u+1] = inp[2u-2] -> cols 0,2,..,222 -> u 1..112
                    if n % 2 == 0:
                        nc.vector.tensor_copy(out=xe_t[:vr, :, 2:114, n],
                                              in_=in_t[:vr, :, 1:224:2])
                        nc.scalar.copy(out=xo_t[:vr, :, 1:113, n],
                                       in_=in_t[:vr, :, 0:223:2])
                    else:
                        nc.scalar.copy(out=xe_t[:vr, :, 2:114, n],
                                       in_=in_t[:vr, :, 1:224:2])
                        nc.vector.tensor_copy(out=xo_t[:vr, :, 1:113, n],
                                              in_=in_t[:vr, :, 0:223:2])
                for (ph_t, rep_d) in ((xe_t, xe_d), (xo_t, xo_d)):
                    dst = _dram_ap(rep_d, r0 * S_ROW,
                                   [[S_ROW, PHW], [1, 3 * NB * PHW]])
                    nc.scalar.dma_start(out=dst, in_=ph_t[:PHW, :, :, :])

            def _bail():
                stg0 = big_p.tile([OC, 14, 56], BF16, tag="bigB", name="bailstg")
                nc.vector.memset(stg0[:, :, :], 0.0)
                for n in range(NB):
                    for qq in range(4):
                        nc.sync.dma_start(out=out_d[n, :, qq * 14:qq * 14 + 14, :],
                                          in_=stg0)

            if stop_after == 'split':
                _bail()
                raise _StopBuild()

            # ---- big tiles ----
            fcomp = big_p.tile([OC, NB, NFC], BF16, tag="bigA")
            for W in wins:
                fhw0 = W.fh * W.fw
                if fhw0 % 2:
                    nc.gpsimd.memset(
                        _sub_ap(fcomp[:], W.foff + fhw0, [[NFC, NB], [1, 1]]),
                        0.0)
            canvas = big_p.tile([OC, NB, CV, CV], BF16, tag="bigB")
            nc.vector.memset(canvas[:, :, 0, :], NEG)
            nc.gpsimd.memset(
                _sub_ap(canvas[:], 0, [[CV * CV, NB], [CV, CV]]), NEG)

            # stats accumulators: one column per (group, image), no chaining
            acc1 = small_p.tile([OC, 64], F32)
            acc2 = small_p.tile([OC, 64], F32)

            # ---- copy-op emitters with engine choice ----
            def pick_engine(n_elems, contig, allow_v=True, dma_ok=False):
                cost = {
                    'v': 120 + n_elems * (0.55 if contig else 1.05),
                    'a': 260 + n_elems * 0.85,
                    'g': 600 + n_elems * 1.40,
                }
                if dma_ok and n_elems >= 500:
                    cost['ds'] = 640 + n_elems * 0.01
                if not allow_v:
                    cost.pop('v')
                e = min(cost, key=lambda k: ecost[k] + cost[k])
                ecost[e] += cost[e]
                return e

            def emit_copy(dst, src, n_elems, contig, allow_v=True, dma_ok=False):
                e = pick_engine(n_elems, contig, allow_v, dma_ok)
                if e == 'v':
                    nc.vector.tensor_copy(out=dst, in_=src)
                elif e == 'a':
                    nc.scalar.copy(out=dst, in_=src)
                elif e == 'g':
                    nc.gpsimd.tensor_copy(out=dst, in_=src)
                else:
                    nc.sync.dma_start(out=dst, in_=src)

            # ---- expansion op emission for one window ----
            def emit_expansion(W, wi):
                if W.row_id:
                    src_t, src_base, src_n, src_row = fcomp, W.foff, NFC, W.fw
                else:
                    rw = rw_p.tile([OC, NB, W.oh, W.fw], BF16, tag="rw",
                                   name=f"rw_{wi}")
                    for (o0, ostep, i0, istep, cnt) in W.rp:
                        dst = _sub_ap(rw[:], o0 * W.fw,
                                      [[W.oh * W.fw, NB],
                                       [ostep * W.fw, cnt], [1, W.fw]])
                        src = _sub_ap(fcomp[:], W.foff + i0 * W.fw,
                                      [[NFC, NB], [istep * W.fw, cnt], [1, W.fw]])
                        emit_copy(dst, src, NB * cnt * W.fw, True,
                                  dma_ok=(ostep == 1 and istep == 1))
                    src_t, src_base, src_n, src_row = rw, 0, W.oh * W.fw, W.fw
                cbase = (1 + W.to) * CV + 1 + W.lo
                for (o0, ostep, i0, istep, cnt) in W.cp:
                    dst = _sub_ap(canvas[:], cbase + o0,
                                  [[CV * CV, NB], [CV, W.oh], [ostep, cnt]])
                    src = _sub_ap(src_t[:], src_base + i0,
                                  [[src_n, NB], [src_row, W.oh], [istep, cnt]])
                    emit_copy(dst, src, NB * W.oh * cnt,
                              ostep == 1 and istep == 1)

            # ---- conv1 + fused stats + expansion ----
            CH_GROUP = 2
            scrA = scr_p.tile([OC, 1024], BF16, tag="poolm0", name="scrA")
            scrB = scr_p.tile([OC, 1024], BF16, tag="poolm1", name="scrB")
            grp_start = 0
            grp_idx = 0
            stats_q = []
            stats_n = [0]

            for wi, W in enumerate(wins):
                q = (wi % 4) if use_tiling else 0
                fhw = W.fh * W.fw
                W3 = W.fw + 3
                li_even = (W.li % 2 == 0)
                base4, base3 = W.li >> 1, (W.li + 1) >> 1
                src4 = xe_d if li_even else xo_d
                src3 = xo_d if li_even else xe_d
                SE = stage_p.tile([128, W.fh * W3 * NB], BF16, tag="se",
                                  name=f"se_{wi}")
                SO = stage_p.tile([128, W.fh * W3 * NB], BF16, tag="so",
                                  name=f"so_{wi}")
                sl_e = SE[32 * q:32 * q + 21, :]
                sl_o = SO[32 * q:32 * q + 21, :]
                dst_e = AP(tensor=sl_e.tensor, offset=sl_e.offset,
                           ap=[list(sl_e.ap[0])] + [[1, W.fh * W3 * NB]])
                dst_o = AP(tensor=sl_o.tensor, offset=sl_o.offset,
                           ap=[list(sl_o.ap[0])] + [[1, W.fh * W3 * NB]])
                nc.sync.dma_start(out=dst_e, in_=_dram_ap(
                    src4, W.ti * S_ROW + base4 * NB,
                    [[S_C, 21], [2 * S_ROW, W.fh], [1, W3 * NB]]))
                nc.scalar.dma_start(out=dst_o, in_=_dram_ap(
                    src3, W.ti * S_ROW + base3 * NB,
                    [[S_C, 21], [2 * S_ROW, W.fh], [1, W3 * NB]]))
                ecost['ds'] += 900
                ecost['a'] += 900
                pss = [psum_p.tile([OC, 2, W.fh, W.fw], F32,
                                   tag="ps", name=f"ps_{wi}_{h}")
                       for h in range(2)]
                tp = (32 * q, 0) if use_tiling else None
                for t in range(7):
                    kxi = t if t < 4 else t - 4
                    sl = sl_e if t < 4 else sl_o
                    for h in range(2):
                        rhs = AP(tensor=sl.tensor,
                                 offset=sl.offset + 2 * h + kxi * NB,
                                 ap=[list(sl.ap[0])] +
                                    [[1, 2], [W3 * NB, W.fh], [NB, W.fw]])
                        nc.tensor.matmul(pss[h][:, :, :, :],
                                         lhsT=w1sb[32 * q:32 * q + 21, t, :],
                                         rhs=rhs, start=(t == 0), stop=(t == 6),
                                         tile_position=tp)
                for h in range(2):
                    dst = _sub_ap(fcomp[:], 2 * h * NFC + W.foff,
                                  [[NFC, 2], [1, fhw]])
                    srcp = pss[h][:, :, :, :].rearrange("p a b c -> p (a b c)")
                    nc.vector.tensor_copy(out=dst, in_=srcp)
                    ecost['v'] += 150 + fhw * 2 * 0.6
                if not no_exp:
                    emit_expansion(W, wi)
                # fused stats for finished window group
                if (not no_stats) and (wi + 1) % CH_GROUP == 0:
                    stats_q.append((grp_start, W.foff + fhw, grp_idx))
                    grp_start = W.foff + fhw
                    grp_idx += 1

                def drain_stats(k):
                    while stats_q and k > 0:
                        g_lo, g_hi, gi = stats_q[0]
                        n = stats_n[0]
                        cw = g_hi - g_lo
                        col = gi * NB + n
                        fslice = _sub_ap(fcomp[:], n * NFC + g_lo, [[1, cw]])
                        nc.vector.scalar_tensor_tensor(
                            out=scrA[:, 0:cw], in0=fslice, scalar=1.0,
                            in1=wdsb[:, g_lo:g_hi],
                            op0=mybir.AluOpType.mult,
                            op1=mybir.AluOpType.mult,
                            accum_out=acc1[:, col:col + 1])
                        nc.vector.scalar_tensor_tensor(
                            out=scrB[:, 0:cw], in0=scrA[:, 0:cw], scalar=1.0,
                            in1=fslice,
                            op0=mybir.AluOpType.mult,
                            op1=mybir.AluOpType.mult,
                            accum_out=acc2[:, col:col + 1])
                        ecost['v'] += 2 * (150 + cw * 1.9)
                        if stats_n[0] == NB - 1:
                            stats_q.pop(0)
                            stats_n[0] = 0
                        else:
                            stats_n[0] += 1
                        k -= 1

                if not no_stats:
                    drain_stats(1 if wi < 14 else 3)

            if not no_stats:
                drain_stats(1000)

            if stop_after == 'conv1':
                _bail()
                raise _StopBuild()

            # ---- BN1: cross-image + cross-core reduction ----
            st1 = small_p.tile([OC, 2], F32)
            nc.vector.tensor_reduce(out=st1[:, 0:1], in_=acc1[:, 0:56],
                                    axis=mybir.AxisListType.X,
                                    op=mybir.AluOpType.add)
            nc.vector.tensor_reduce(out=st1[:, 1:2], in_=acc2[:, 0:56],
                                    axis=mybir.AxisListType.X,
                                    op=mybir.AluOpType.add)
            cc1_in = dram_p.tile([OC, 2], F32)
            cc1_out = dram_p.tile([OC, 2], F32)
            nc.sync.dma_start(out=cc1_in[:, :], in_=st1)
            if no_cc:
                nc.sync.dma_start(out=cc1_out[:, :], in_=cc1_in[:, :])
            else:
                nc.gpsimd.collective_compute(
                    "AllReduce", mybir.AluOpType.add,
                    replica_groups=[list(range(N_CORES))],
                    ins=[cc1_in.opt()], outs=[cc1_out.opt()])
            sums1 = small_p.tile([OC, 2], F32)
            nc.sync.dma_start(out=sums1, in_=cc1_out[:, :])

            epst = small_p.tile([OC, 1], F32)
            nc.vector.memset(epst, EPS)

            def bn_coefs(sums, gb, npix, name):
                m = small_p.tile([OC, 1], F32, tag=f"m_{name}", name=f"m_{name}")
                v = small_p.tile([OC, 1], F32, tag=f"v_{name}", name=f"v_{name}")
                a = small_p.tile([OC, 1], F32, tag=f"a_{name}", name=f"a_{name}")
                c = small_p.tile([OC, 1], F32, tag=f"c_{name}", name=f"c_{name}")
                nc.vector.tensor_scalar_mul(m, sums[:, 0:1], 1.0 / npix)
                nc.vector.tensor_scalar_mul(v, sums[:, 1:2], 1.0 / npix)
                mm = small_p.tile([OC, 1], F32, tag=f"mm_{name}", name=f"mm_{name}")
                nc.vector.tensor_mul(mm, m, m)
                nc.vector.tensor_sub(v, v, mm)
                nc.scalar.activation(out=a, in_=v,
                                     func=mybir.ActivationFunctionType.Sqrt,
                                     bias=epst[:, 0:1])
                nc.vector.reciprocal(out=a, in_=a)
                nc.vector.tensor_mul(a, a, gb[:, 0:1])
                nc.vector.tensor_mul(c, a, m)
                nc.vector.tensor_sub(c, gb[:, 1:2], c)
                return a, c

            if stop_after == 'stats1':
                _bail()
                raise _StopBuild()

            # ---- maxpool (raw) into hpad, then BN1+ReLU in place ----
            hpad = big_p.tile([OC, NB, 58, 58], BF16, tag="bigA")
            nc.gpsimd.memset(
                _sub_ap(hpad[:], 0, [[58 * 58, NB], [57 * 58, 2], [1, 58]]), 0.0)
            nc.gpsimd.memset(
                _sub_ap(hpad[:], 0, [[58 * 58, NB], [58, 58], [57, 2]]), 0.0)
            for n in range(NB):
                for cb in range(0, 56, 28):
                    pt = scr_p.tile([OC, 28, CV], BF16, tag="poolsc",
                                    name=f"pt_{n}_{cb}")
                    cvs = canvas[:, n, :, :]
                    r0 = 2 * cb
                    nc.vector.tensor_max(pt[:, :, :],
                                         cvs[:, r0 + 0:r0 + 56:2, :],
                                         cvs[:, r0 + 1:r0 + 57:2, :])
                    nc.vector.tensor_max(pt[:, :, :], pt[:, :, :],
                                         cvs[:, r0 + 2:r0 + 57:2, :])
                    hint = hpad[:, n, 1 + cb:1 + cb + 28, 1:57]
                    nc.vector.tensor_max(hint, pt[:, :, 0:111:2],
                                         pt[:, :, 1:112:2])
                    nc.vector.tensor_max(hint, hint, pt[:, :, 2:113:2])
            # bn coefs computed while pool drains; DVE reciprocal sits after
            # all pool maxes so it never blocks them
            a1, c1 = bn_coefs(sums1, gb1, NPIX1, "bn1")
            for n in range(NB):
                hint = hpad[:, n, 1:57, 1:57]
                nc.scalar.activation(
                    out=hint, in_=hint,
                    func=mybir.ActivationFunctionType.Relu,
                    scale=a1[:, 0:1], bias=c1[:, 0:1])

            if stop_after == 'pool':
                _bail()
                raise _StopBuild()

            # ---- conv2 (9 shifted matmuls, K=128) + fused stats2 ----
            c2raw = big_p.tile([OC, NB, 56, 56], BF16, tag="c2raw", name="c2raw")
            sxp = small_p.tile([OC, 28], F32)   # per-chunk sum(x)
            sq28 = small_p.tile([OC, 28], F32)  # per-chunk sum(x^2)
            ci = 0
            for n in range(NB):
                for y0 in range(0, 56, 8):
                    ps = psum_p.tile([OC, 8, 56], F32, tag="ps",
                                     name=f"ps2_{n}_{y0}")
                    for tap in range(9):
                        ky, kx = tap // 3, tap % 3
                        rhs = _sub_ap(hpad[:], n * 58 * 58 + (y0 + ky) * 58 + kx,
                                      [[58, 8], [1, 56]])
                        nc.tensor.matmul(ps[:, :, :], lhsT=w2sb[:, tap, :],
                                         rhs=rhs, start=(tap == 0),
                                         stop=(tap == 8))
                    nc.scalar.activation(
                        out=c2raw[:, n, y0:y0 + 8, :], in_=ps[:, :, :],
                        func=mybir.ActivationFunctionType.Identity,
                        accum_out=sxp[:, ci:ci + 1])
                    ci += 1
                # scrC reuses the pool scratch slot (pool is drained before
                # these issue on their queues)
                scrC = scr_p.tile([OC, 28 * 112], BF16, tag="poolsc",
                                  name=f"scrC_{n}")
                cn = c2raw[:, n, :, :].rearrange("p a b -> p (a b)")
                nc.scalar.activation(
                    out=scrC[:, 0:3136], in_=cn,
                    func=mybir.ActivationFunctionType.Square,
                    accum_out=sq28[:, n:n + 1])

            if stop_after == 'conv2':
                _bail()
                raise _StopBuild()

            # ---- BN2 stats + allreduce ----
            st2 = small_p.tile([OC, 2], F32)
            nc.vector.tensor_reduce(out=st2[:, 0:1], in_=sxp[:, :],
                                    axis=mybir.AxisListType.X,
                                    op=mybir.AluOpType.add)
            nc.vector.tensor_reduce(out=st2[:, 1:2], in_=sq28[:, 0:NB],
                                    axis=mybir.AxisListType.X,
                                    op=mybir.AluOpType.add)
            cc2_in = dram_p.tile([OC, 2], F32)
            cc2_out = dram_p.tile([OC, 2], F32)
            nc.sync.dma_start(out=cc2_in[:, :], in_=st2)
            if no_cc:
                nc.sync.dma_start(out=cc2_out[:, :], in_=cc2_in[:, :])
            else:
                nc.gpsimd.collective_compute(
                    "AllReduce", mybir.AluOpType.add,
                    replica_groups=[list(range(N_CORES))],
                    ins=[cc2_in.opt()], outs=[cc2_out.opt()])
            sums2 = small_p.tile([OC, 2], F32)
            nc.sync.dma_start(out=sums2, in_=cc2_out[:, :])
            a2, c2 = bn_coefs(sums2, gb2, NPIX2, "bn2")

            # ---- final: relu(a2*x + c2) -> out ----
            ostg = big_p.tile([OC, NB, 56, 56], BF16, tag="bigB", name="ostg")
            for n in range(NB):
                for yb in range(0, 56, 28):
                    oslice = ostg[:, n, yb:yb + 28, :]
                    cslice = c2raw[:, n, yb:yb + 28, :]
                    if n % 2 == 0:
                        nc.scalar.activation(
                            out=oslice, in_=cslice,
                            func=mybir.ActivationFunctionType.Relu,
                            scale=a2[:, 0:1], bias=c2[:, 0:1])
                    else:
                        nc.vector.tensor_scalar(
                            out=oslice, in0=cslice,
                            scalar1=a2[:, 0:1], scalar2=c2[:, 0:1],
                            op0=mybir.AluOpType.mult,
                            op1=mybir.AluOpType.add)
                        nc.vector.tensor_scalar_max(oslice, oslice, 0.0)
                    dma_eng = nc.sync if n < 2 else nc.gpsimd
                    dma_eng.dma_start(out=out_d[n, :, yb:yb + 28, :],
                                      in_=oslice)

        except _StopBuild:
            pass

    nc.compile()
    return nc


_CACHE = {}


def _get_nc(conv_w, gamma1, beta1, conv2_w, gamma2, beta2):
    import hashlib
    key = hashlib.sha256(b''.join(np.ascontiguousarray(a).tobytes()
                                  for a in (conv_w, gamma1, beta1, conv2_w,
                                            gamma2, beta2))).hexdigest()
    if key not in _CACHE:
        _CACHE[key] = _build(conv_w, gamma1, beta1, conv2_w, gamma2, beta2)
    return _CACHE[key]


def kernel(inp, conv_w, gamma1, beta1, conv2_w, gamma2, beta2):
    inp = np.asarray(inp, np.float32)
    nc = _get_nc(np.asarray(conv_w, np.float32), np.asarray(gamma1, np.float32),
                 np.asarray(beta1, np.float32), np.asarray(conv2_w, np.float32),
                 np.asarray(gamma2, np.float32), np.asarray(beta2, np.float32))
    in_maps = [{"inp": np.ascontiguousarray(inp[i * NB:(i + 1) * NB])}
               for i in range(N_CORES)]
    res = run_bass_kernel_spmd(nc, in_maps, list(range(N_CORES)))
    return np.concatenate([np.asarray(res.results[i]["out"], np.float32)
                           for i in range(N_CORES)], axis=0)


# revision 24
# speedup vs baseline: 1.1365x; 1.1365x over previous
"""Trainium2 Bass kernel for nn_COGV1 (foveated multi-scale conv + BN + pool + conv).

v2: 4-image-merged staging DMAs, PE row-tiling (32x128 quadrants) for conv1,
pool-before-BN reorder (raw expansion, -inf pad ring, BN+ReLU fused after
pool), BN stats fused into conv1 via tensor_tensor_reduce, cost-model-driven
engine placement for expansion copies.
"""
import numpy as np

try:
    import concourse.bass as bass  # noqa: F401
except ImportError:  # grading env may not have the repo on sys.path yet
    import sys
    for _p in ('/opt/trn_rl_repo', '/root/.axon_site/_ro/trn_rl_repo'):
        if _p not in sys.path:
            sys.path.insert(0, _p)
    import concourse.bass as bass  # noqa: F401

import concourse.bacc as bacc
import concourse.tile as tile
from concourse import mybir
from concourse.bass import AP
from concourse.bass_utils import run_bass_kernel_spmd

F32 = mybir.dt.float32
BF16 = mybir.dt.bfloat16

OC = 128; KS = 7; STRIDE = 2; IMG = 224; NS = 7; PAD = 3; EPS = 1e-5
OUT_HW = 112
XPAD = 230   # padded image size
PHW = 115    # phase image width
N_CORES = 8
NB = 4       # images per core
NPIX1 = 32 * OUT_HW * OUT_HW   # BN1 count (full batch)
NPIX2 = 32 * 56 * 56           # BN2 count
CV = 113     # canvas with 1-px -inf pad at top/left
NEG = -1.0e30

# DRAM phase layout [row 230][c 3][u 115][n 4] (n innermost for long DMA lines)
S_U = NB                  # 4
S_C = PHW * NB            # 460
S_ROW = 3 * S_C           # 1380
REP_SZ = XPAD * S_ROW


class _Win:
    pass


class _StopBuild(Exception):
    pass


def _decompose(idx, n_out, n_in):
    """pieces (o0, ostep, i0, istep, cnt): out[o0+k*ostep] = in[i0+k*istep]."""
    idx = np.asarray(idx)
    runs = []
    j = 0
    while j < n_out:
        j0 = j
        while j + 1 < n_out and idx[j + 1] == idx[j] + 1:
            j += 1
        runs.append((j0, 1, int(idx[j0]), 1, j - j0 + 1))
        j += 1
    g = int(np.gcd(n_out, n_in))
    p_out, p_in = n_out // g, n_in // g
    per = [(u, p_out, int(idx[u]), p_in, g) for u in range(p_out)]
    ok = all(idx[u + k * p_out] == idx[u] + k * p_in
             for u in range(p_out) for k in range(g))
    if ok and len(per) < len(runs):
        return per
    return runs


def _windows():
    scales = np.linspace(2.0, 1.0, NS, dtype=np.float32)
    borders = np.linspace(0, IMG // 2, NS + 1).astype(np.int64)
    iw = np.empty((NS, 4, 4), dtype=np.int64)
    for s in range(NS):
        a, b = borders[s], borders[s + 1]
        c, d = IMG - b, IMG - a
        iw[s] = np.array([[a, a, b, c], [b, a, d, b], [c, b, d, d], [a, c, c, d]])
    ow = iw // STRIDE
    iwp = iw + PAD * 2
    iwp = np.stack([(iwp[s].astype(np.float32) / scales[s]).astype(np.int64)
                    for s in range(NS)])
    iwp[:, :, :2] -= PAD
    iwp[:, :, 2:] += PAD
    wins = []
    foff = 0
    for s in range(NS):
        for w in range(4):
            W = _Win()
            W.ti, W.li, W.bi, W.ri = (int(v) for v in iwp[s, w])
            W.fh = (W.bi - W.ti - KS) // STRIDE + 1
            W.fw = (W.ri - W.li - KS) // STRIDE + 1
            W.to, W.lo, W.bo, W.ro = (int(v) for v in ow[s, w])
            W.oh, W.ow = W.bo - W.to, W.ro - W.lo
            W.ih = (np.arange(W.oh) * W.fh) // W.oh
            W.iw = (np.arange(W.ow) * W.fw) // W.ow
            W.wrow = np.bincount(W.ih, minlength=W.fh)
            W.wcol = np.bincount(W.iw, minlength=W.fw)
            W.rp = _decompose(W.ih, W.oh, W.fh)
            W.cp = _decompose(W.iw, W.ow, W.fw)
            W.row_id = (W.oh == W.fh and len(W.rp) == 1
                        and tuple(W.rp[0]) == (0, 1, 0, 1, W.oh))
            W.foff = foff; foff += W.fh * W.fw + (W.fh * W.fw) % 2
            wins.append(W)
    return wins, foff


def _sub_ap(tile_ap, extra_off, dims):
    """AP over a tile's tensor: keep partition dim, custom free dims."""
    return AP(tensor=tile_ap.tensor, offset=tile_ap.offset + extra_off,
              ap=[list(tile_ap.ap[0])] + [list(d) for d in dims])


def _dram_ap(handle, off, dims):
    return AP(tensor=handle, offset=off, ap=[list(d) for d in dims])


def _build(conv_w, gamma1, beta1, conv2_w, gamma2, beta2,
           stop_after=None, no_cc=False, use_tiling=True,
           no_exp=False, no_stats=False):
    import ml_dtypes
    import contextlib
    bf = lambda x: np.asarray(x, ml_dtypes.bfloat16)
    wins, NFC = _windows()

    nc = bacc.Bacc("TRN2", target_bir_lowering=False, debug=False,
                   num_devices=N_CORES)
    inp_d = nc.declare_dram_parameter("inp", [NB, 3, IMG, IMG], F32, isOutput=False)
    out_d = nc.declare_dram_parameter("out", [NB, OC, 56, 56], BF16, isOutput=True)

    # conv1 weights: rows k = ky*3 + c; taps 0..3 = kx 0,2,4,6; 4..6 = kx 1,3,5
    # replicated into 4 partition quadrants for PE row tiling.
    l1 = np.zeros((128, 7, OC), np.float32)
    for ky in range(KS):
        for c in range(3):
            for q in range(4):
                for t in range(4):
                    l1[32 * q + ky * 3 + c, t] = conv_w[:, c, ky, 2 * t]
                for t in range(3):
                    l1[32 * q + ky * 3 + c, 4 + t] = conv_w[:, c, ky, 2 * t + 1]
    l1_d = nc.inline_tensor(bf(l1), name="l1")
    w2 = np.transpose(conv2_w, (1, 2, 3, 0)).reshape(OC, 9, OC)
    w2_d = nc.inline_tensor(bf(w2), name="w2")
    wd = np.zeros(NFC, np.float32)
    for W in wins:
        wd[W.foff:W.foff + W.fh * W.fw] = np.outer(W.wrow, W.wcol).reshape(-1)
    wd_d = nc.inline_tensor(bf(wd), name="wd")
    gb1_d = nc.inline_tensor(np.stack([gamma1, beta1], 1).astype(np.float32), name="gb1")
    gb2_d = nc.inline_tensor(np.stack([gamma2, beta2], 1).astype(np.float32), name="gb2")

    xe_d = nc.dram_tensor("xe_rep", [REP_SZ + 64], BF16)
    xo_d = nc.dram_tensor("xo_rep", [REP_SZ + 64], BF16)

    # running engine load (ns) for copy placement; pre-biased for known
    # conv1-phase work issued outside the greedy (stats+copies on v, etc).
    ecost = {'v': 0.0, 'a': 0.0, 'g': 0.0, 'ds': 0.0}

    with tile.TileContext(nc) as tc:
        ctx = contextlib.ExitStack()
        try:
          with ctx:
            consts_p = ctx.enter_context(tc.tile_pool(name="consts", bufs=1))
            stage_p = ctx.enter_context(tc.tile_pool(name="stage", bufs=3))
            scr_p = ctx.enter_context(tc.tile_pool(name="scr", bufs=1))
            psum_p = ctx.enter_context(tc.tile_pool(name="psum", bufs=8, space="PSUM"))
            big_p = ctx.enter_context(tc.tile_pool(name="big", bufs=1))
            rw_p = ctx.enter_context(tc.tile_pool(name="rwp", bufs=2))
            small_p = ctx.enter_context(tc.tile_pool(name="small", bufs=1))
            dram_p = ctx.enter_context(tc.tile_pool(name="dramp", bufs=1, space="DRAM"))

            # ---- constants ----
            w1sb = consts_p.tile([128, 7, OC], BF16)
            w2sb = consts_p.tile([OC, 9, OC], BF16)
            gb1 = consts_p.tile([OC, 2], F32)
            gb2 = consts_p.tile([OC, 2], F32)
            # wdsb shares the (later) c2raw slot: disjoint live ranges
            wdsb = big_p.tile([OC, NFC], BF16, tag="c2raw", name="wdsb")
            nc.scalar.dma_start(out=w1sb, in_=l1_d[:, :, :])
            nc.scalar.dma_start(out=w2sb, in_=w2_d[:, :, :])
            nc.scalar.dma_start(out=wdsb, in_=_dram_ap(wd_d, 0, [[0, OC], [1, NFC]]))
            nc.scalar.dma_start(out=gb1, in_=gb1_d[:, :])
            nc.scalar.dma_start(out=gb2, in_=gb2_d[:, :])

            # ---- phase split: [row][c][n][u] bf16 replicas in DRAM ----
            for half in range(2):
                r0 = half * PHW
                xe_t = stage_p.tile([PHW, 3, PHW, NB], BF16, tag="so",
                                    name=f"xe_t{half}")
                xo_t = stage_p.tile([PHW, 3, PHW, NB], BF16, tag="so",
                                    name=f"xo_t{half}")
                nc.vector.memset(xe_t[:, :, :, :], 0.0)
                nc.vector.memset(xo_t[:, :, :, :], 0.0)
                for n in range(NB):
                    in_t = stage_p.tile([128, 3, IMG], F32, tag="se",
                                        name=f"in_t{half}_{n}")
                    lo = max(0, r0 - PAD)
                    hi = min(IMG, r0 + PHW - PAD)
                    p_lo = lo - (r0 - PAD)
                    if p_lo > 0:
                        nc.vector.memset(in_t[0:p_lo, :, :], 0.0)
                    vr = p_lo + hi - lo   # valid rows; rows beyond stay 0 in xe/xo
                    (nc.sync if n % 2 == 0 else nc.scalar).dma_start(
                        out=in_t[p_lo:p_lo + hi - lo, :, :],
                        in_=inp_d[n, :, lo:hi, :].transpose([1, 0, 2]))
                    ecost['ds' if n % 2 == 0 else 'a'] += 1800
                    # xe[u] = xpad[2u]   = inp[2u-3] -> cols 1,3,..,223 -> u 2..113
                    # xo[u] = xpad[2u+1] = inp[2u-2] -> cols 0,2,..,222 -> u 1..112
                    if n % 2 == 0:
                        nc.vector.tensor_copy(out=xe_t[:vr, :, 2:114, n],
                                              in_=in_t[:vr, :, 1:224:2])
                        nc.scalar.copy(out=xo_t[:vr, :, 1:113, n],
                                       in_=in_t[:vr, :, 0:223:2])
                    else:
                        nc.scalar.copy(out=xe_t[:vr, :, 2:114, n],
                                       in_=in_t[:vr, :, 1:224:2])
                        nc.vector.tensor_copy(out=xo_t[:vr, :, 1:113, n],
                                              in_=in_t[:vr, :, 0:223:2])
                for (ph_t, rep_d) in ((xe_t, xe_d), (xo_t, xo_d)):
                    dst = _dram_ap(rep_d, r0 * S_ROW,
                                   [[S_ROW, PHW], [1, 3 * NB * PHW]])
                    nc.scalar.dma_start(out=dst, in_=ph_t[:PHW, :, :, :])

            def _bail():
                stg0 = big_p.tile([OC, 14, 56], BF16, tag="bigB", name="bailstg")
                nc.vector.memset(stg0[:, :, :], 0.0)
                for n in range(NB):
                    for qq in range(4):
                        nc.sync.dma_start(out=out_d[n, :, qq * 14:qq * 14 + 14, :],
                                          in_=stg0)

            if stop_after == 'split':
                _bail()
                raise _StopBuild()

            # ---- big tiles ----
            fcomp = big_p.tile([OC, NB, NFC], BF16, tag="bigA")
            for W in wins:
                fhw0 = W.fh * W.fw
                if fhw0 % 2:
                    nc.gpsimd.memset(
                        _sub_ap(fcomp[:], W.foff + fhw0, [[NFC, NB], [1, 1]]),
                        0.0)
            canvas = big_p.tile([OC, NB, CV, CV], BF16, tag="bigB")
            nc.vector.memset(canvas[:, :, 0, :], NEG)
            nc.gpsimd.memset(
                _sub_ap(canvas[:], 0, [[CV * CV, NB], [CV, CV]]), NEG)

            # stats accumulators: one column per (group, image), no chaining
            acc1 = small_p.tile([OC, 64], F32)
            acc2 = small_p.tile([OC, 64], F32)

            # ---- copy-op emitters with engine choice ----
            def pick_engine(n_elems, contig, allow_v=True, dma_ok=False):
                cost = {
                    'v': 120 + n_elems * (0.55 if contig else 1.05),
                    'a': 260 + n_elems * 0.85,
                    'g': 600 + n_elems * 1.40,
                }
                if dma_ok and n_elems >= 500:
                    cost['ds'] = 640 + n_elems * 0.01
                if not allow_v:
                    cost.pop('v')
                e = min(cost, key=lambda k: ecost[k] + cost[k])
                ecost[e] += cost[e]
                return e

            def emit_copy(dst, src, n_elems, contig, allow_v=True, dma_ok=False):
                e = pick_engine(n_elems, contig, allow_v, dma_ok)
                if e == 'v':
                    nc.vector.tensor_copy(out=dst, in_=src)
                elif e == 'a':
                    nc.scalar.copy(out=dst, in_=src)
                elif e == 'g':
                    nc.gpsimd.tensor_copy(out=dst, in_=src)
                else:
                    nc.sync.dma_start(out=dst, in_=src)

            # ---- expansion op emission for one window ----
            def emit_expansion(W, wi):
                if W.row_id:
                    src_t, src_base, src_n, src_row = fcomp, W.foff, NFC, W.fw
                else:
                    rw = rw_p.tile([OC, NB, W.oh, W.fw], BF16, tag="rw",
                                   name=f"rw_{wi}")
                    for (o0, ostep, i0, istep, cnt) in W.rp:
                        dst = _sub_ap(rw[:], o0 * W.fw,
                                      [[W.oh * W.fw, NB],
                                       [ostep * W.fw, cnt], [1, W.fw]])
                        src = _sub_ap(fcomp[:], W.foff + i0 * W.fw,
                                      [[NFC, NB], [istep * W.fw, cnt], [1, W.fw]])
                        emit_copy(dst, src, NB * cnt * W.fw, True,
                                  dma_ok=(ostep == 1 and istep == 1))
                    src_t, src_base, src_n, src_row = rw, 0, W.oh * W.fw, W.fw
                cbase = (1 + W.to) * CV + 1 + W.lo
                for (o0, ostep, i0, istep, cnt) in W.cp:
                    dst = _sub_ap(canvas[:], cbase + o0,
                                  [[CV * CV, NB], [CV, W.oh], [ostep, cnt]])
                    src = _sub_ap(src_t[:], src_base + i0,
                                  [[src_n, NB], [src_row, W.oh], [istep, cnt]])
                    emit_copy(dst, src, NB * W.oh * cnt,
                              ostep == 1 and istep == 1)

            # ---- conv1 + fused stats + expansion ----
            CH_GROUP = 2
            scrA = scr_p.tile([OC, 1024], BF16, tag="poolm0", name="scrA")
            scrB = scr_p.tile([OC, 1024], BF16, tag="poolm1", name="scrB")
            grp_start = 0
            grp_idx = 0
            stats_q = []
            stats_n = [0]

            for wi, W in enumerate(wins):
                q = (wi % 4) if use_tiling else 0
                fhw = W.fh * W.fw
                W3 = W.fw + 3
                li_even = (W.li % 2 == 0)
                base4, base3 = W.li >> 1, (W.li + 1) >> 1
                src4 = xe_d if li_even else xo_d
                src3 = xo_d if li_even else xe_d
                SE = stage_p.tile([128, W.fh * W3 * NB], BF16, tag="se",
                                  name=f"se_{wi}")
                SO = stage_p.tile([128, W.fh * W3 * NB], BF16, tag="so",
                                  name=f"so_{wi}")
                sl_e = SE[32 * q:32 * q + 21, :]
                sl_o = SO[32 * q:32 * q + 21, :]
                dst_e = AP(tensor=sl_e.tensor, offset=sl_e.offset,
                           ap=[list(sl_e.ap[0])] + [[1, W.fh * W3 * NB]])
                dst_o = AP(tensor=sl_o.tensor, offset=sl_o.offset,
                           ap=[list(sl_o.ap[0])] + [[1, W.fh * W3 * NB]])
                nc.sync.dma_start(out=dst_e, in_=_dram_ap(
                    src4, W.ti * S_ROW + base4 * NB,
                    [[S_C, 21], [2 * S_ROW, W.fh], [1, W3 * NB]]))
                nc.scalar.dma_start(out=dst_o, in_=_dram_ap(
                    src3, W.ti * S_ROW + base3 * NB,
                    [[S_C, 21], [2 * S_ROW, W.fh], [1, W3 * NB]]))
                ecost['ds'] += 900
                ecost['a'] += 900
                pss = [psum_p.tile([OC, 2, W.fh, W.fw], F32,
                                   tag="ps", name=f"ps_{wi}_{h}")
                       for h in range(2)]
                tp = (32 * q, 0) if use_tiling else None
                for t in range(7):
                    kxi = t if t < 4 else t - 4
                    sl = sl_e if t < 4 else sl_o
                    for h in range(2):
                        rhs = AP(tensor=sl.tensor,
                                 offset=sl.offset + 2 * h + kxi * NB,
                                 ap=[list(sl.ap[0])] +
                                    [[1, 2], [W3 * NB, W.fh], [NB, W.fw]])
                        nc.tensor.matmul(pss[h][:, :, :, :],
                                         lhsT=w1sb[32 * q:32 * q + 21, t, :],
                                         rhs=rhs, start=(t == 0), stop=(t == 6),
                                         tile_position=tp)
                for h in range(2):
                    dst = _sub_ap(fcomp[:], 2 * h * NFC + W.foff,
                                  [[NFC, 2], [1, fhw]])
                    srcp = pss[h][:, :, :, :].rearrange("p a b c -> p (a b c)")
                    nc.vector.tensor_copy(out=dst, in_=srcp)
                    ecost['v'] += 150 + fhw * 2 * 0.6
                if not no_exp:
                    emit_expansion(W, wi)
                # fused stats for finished window group
                if (not no_stats) and (wi + 1) % CH_GROUP == 0:
                    stats_q.append((grp_start, W.foff + fhw, grp_idx))
                    grp_start = W.foff + fhw
                    grp_idx += 1

                def drain_stats(k):
                    while stats_q and k > 0:
                        g_lo, g_hi, gi = stats_q[0]
                        n = stats_n[0]
                        cw = g_hi - g_lo
                        col = gi * NB + n
                        fslice = _sub_ap(fcomp[:], n * NFC + g_lo, [[1, cw]])
                        nc.vector.scalar_tensor_tensor(
                            out=scrA[:, 0:cw], in0=fslice, scalar=1.0,
                            in1=wdsb[:, g_lo:g_hi],
                            op0=mybir.AluOpType.mult,
                            op1=mybir.AluOpType.mult,
                            accum_out=acc1[:, col:col + 1])
                        nc.vector.scalar_tensor_tensor(
                            out=scrB[:, 0:cw], in0=scrA[:, 0:cw], scalar=1.0,
                            in1=fslice,
                            op0=mybir.AluOpType.mult,
                            op1=mybir.AluOpType.mult,
                            accum_out=acc2[:, col:col + 1])
                        ecost['v'] += 2 * (150 + cw * 1.9)
                        if stats_n[0] == NB - 1:
                            stats_q.pop(0)
                            stats_n[0] = 0
                        else:
                            stats_n[0] += 1
                        k -= 1

                if not no_stats:
                    drain_stats(1 if wi < 14 else 3)

            if not no_stats:
                drain_stats(1000)

            # hpad created before the AR so its ring memsets don't queue
            # behind the collective
            hpad = big_p.tile([OC, NB, 58, 58], BF16, tag="bigA")
            nc.vector.memset(
                _sub_ap(hpad[:], 0, [[58 * 58, NB], [57 * 58, 2], [1, 58]]), 0.0)
            nc.vector.memset(
                _sub_ap(hpad[:], 0, [[58 * 58, NB], [58, 58], [57, 2]]), 0.0)

            if stop_after == 'conv1':
                _bail()
                raise _StopBuild()

            # ---- BN1: cross-image + cross-core reduction ----
            st1 = small_p.tile([OC, 2], F32)
            nc.vector.tensor_reduce(out=st1[:, 0:1], in_=acc1[:, 0:56],
                                    axis=mybir.AxisListType.X,
                                    op=mybir.AluOpType.add)
            nc.vector.tensor_reduce(out=st1[:, 1:2], in_=acc2[:, 0:56],
                                    axis=mybir.AxisListType.X,
                                    op=mybir.AluOpType.add)
            cc1_in = dram_p.tile([OC, 2], F32)
            cc1_out = dram_p.tile([OC, 2], F32)
            nc.sync.dma_start(out=cc1_in[:, :], in_=st1)
            if no_cc:
                nc.sync.dma_start(out=cc1_out[:, :], in_=cc1_in[:, :])
            else:
                nc.gpsimd.collective_compute(
                    "AllReduce", mybir.AluOpType.add,
                    replica_groups=[list(range(N_CORES))],
                    ins=[cc1_in.opt()], outs=[cc1_out.opt()])
            sums1 = small_p.tile([OC, 2], F32)
            nc.sync.dma_start(out=sums1, in_=cc1_out[:, :])

            epst = small_p.tile([OC, 1], F32)
            nc.vector.memset(epst, EPS)

            def bn_coefs(sums, gb, npix, name):
                m = small_p.tile([OC, 1], F32, tag=f"m_{name}", name=f"m_{name}")
                v = small_p.tile([OC, 1], F32, tag=f"v_{name}", name=f"v_{name}")
                a = small_p.tile([OC, 1], F32, tag=f"a_{name}", name=f"a_{name}")
                c = small_p.tile([OC, 1], F32, tag=f"c_{name}", name=f"c_{name}")
                nc.vector.tensor_scalar_mul(m, sums[:, 0:1], 1.0 / npix)
                nc.vector.tensor_scalar_mul(v, sums[:, 1:2], 1.0 / npix)
                mm = small_p.tile([OC, 1], F32, tag=f"mm_{name}", name=f"mm_{name}")
                nc.vector.tensor_mul(mm, m, m)
                nc.vector.tensor_sub(v, v, mm)
                nc.scalar.activation(out=a, in_=v,
                                     func=mybir.ActivationFunctionType.Sqrt,
                                     bias=epst[:, 0:1])
                nc.vector.reciprocal(out=a, in_=a)
                nc.vector.tensor_mul(a, a, gb[:, 0:1])
                nc.vector.tensor_mul(c, a, m)
                nc.vector.tensor_sub(c, gb[:, 1:2], c)
                return a, c

            if stop_after == 'stats1':
                _bail()
                raise _StopBuild()

            # ---- maxpool (raw) into hpad, then BN1+ReLU in place ----
            for n in range(NB):
                for cb in range(0, 56, 28):
                    pt = scr_p.tile([OC, 28, CV], BF16, tag="poolsc",
                                    name=f"pt_{n}_{cb}")
                    cvs = canvas[:, n, :, :]
                    r0 = 2 * cb
                    nc.vector.tensor_max(pt[:, :, :],
                                         cvs[:, r0 + 0:r0 + 56:2, :],
                                         cvs[:, r0 + 1:r0 + 57:2, :])
                    nc.vector.tensor_max(pt[:, :, :], pt[:, :, :],
                                         cvs[:, r0 + 2:r0 + 57:2, :])
                    hint = hpad[:, n, 1 + cb:1 + cb + 28, 1:57]
                    nc.vector.tensor_max(hint, pt[:, :, 0:111:2],
                                         pt[:, :, 1:112:2])
                    nc.vector.tensor_max(hint, hint, pt[:, :, 2:113:2])
            # bn coefs computed while pool drains; DVE reciprocal sits after
            # all pool maxes so it never blocks them
            a1, c1 = bn_coefs(sums1, gb1, NPIX1, "bn1")
            for n in range(NB):
                hint = hpad[:, n, 1:57, 1:57]
                nc.scalar.activation(
                    out=hint, in_=hint,
                    func=mybir.ActivationFunctionType.Relu,
                    scale=a1[:, 0:1], bias=c1[:, 0:1])

            if stop_after == 'pool':
                _bail()
                raise _StopBuild()

            # ---- conv2 (9 shifted matmuls, K=128) + fused stats2 ----
            c2raw = big_p.tile([OC, NB, 56, 56], BF16, tag="c2raw", name="c2raw")
            sxp = small_p.tile([OC, 28], F32)   # per-chunk sum(x)
            sq28 = small_p.tile([OC, 28], F32)  # per-chunk sum(x^2)
            ci = 0
            for n in range(NB):
                for y0 in range(0, 56, 8):
                    ps = psum_p.tile([OC, 8, 56], F32, tag="ps",
                                     name=f"ps2_{n}_{y0}")
                    for tap in range(9):
                        ky, kx = tap // 3, tap % 3
                        rhs = _sub_ap(hpad[:], n * 58 * 58 + (y0 + ky) * 58 + kx,
                                      [[58, 8], [1, 56]])
                        nc.tensor.matmul(ps[:, :, :], lhsT=w2sb[:, tap, :],
                                         rhs=rhs, start=(tap == 0),
                                         stop=(tap == 8))
                    nc.scalar.activation(
                        out=c2raw[:, n, y0:y0 + 8, :], in_=ps[:, :, :],
                        func=mybir.ActivationFunctionType.Identity,
                        accum_out=sxp[:, ci:ci + 1])
                    ci += 1
                # scrC reuses the pool scratch slot (pool is drained before
                # these issue on their queues)
                scrC = scr_p.tile([OC, 28 * 112], BF16, tag="poolsc",
                                  name=f"scrC_{n}")
                cn = c2raw[:, n, :, :].rearrange("p a b -> p (a b)")
                nc.scalar.activation(
                    out=scrC[:, 0:3136], in_=cn,
                    func=mybir.ActivationFunctionType.Square,
                    accum_out=sq28[:, n:n + 1])

            if stop_after == 'conv2':
                _bail()
                raise _StopBuild()

            # ---- BN2 stats + allreduce ----
            st2 = small_p.tile([OC, 2], F32)
            nc.vector.tensor_reduce(out=st2[:, 0:1], in_=sxp[:, :],
                                    axis=mybir.AxisListType.X,
                                    op=mybir.AluOpType.add)
            nc.vector.tensor_reduce(out=st2[:, 1:2], in_=sq28[:, 0:NB],
                                    axis=mybir.AxisListType.X,
                                    op=mybir.AluOpType.add)
            cc2_in = dram_p.tile([OC, 2], F32)
            cc2_out = dram_p.tile([OC, 2], F32)
            nc.sync.dma_start(out=cc2_in[:, :], in_=st2)
            if no_cc:
                nc.sync.dma_start(out=cc2_out[:, :], in_=cc2_in[:, :])
            else:
                nc.gpsimd.collective_compute(
                    "AllReduce", mybir.AluOpType.add,
                    replica_groups=[list(range(N_CORES))],
                    ins=[cc2_in.opt()], outs=[cc2_out.opt()])
            sums2 = small_p.tile([OC, 2], F32)
            nc.sync.dma_start(out=sums2, in_=cc2_out[:, :])
            a2, c2 = bn_coefs(sums2, gb2, NPIX2, "bn2")

            # ---- final: relu(a2*x + c2) -> out ----
            ostg = big_p.tile([OC, NB, 56, 56], BF16, tag="bigB", name="ostg")
            for n in range(NB):
                for yb in range(0, 56, 28):
                    oslice = ostg[:, n, yb:yb + 28, :]
                    cslice = c2raw[:, n, yb:yb + 28, :]
                    if n % 2 == 0:
                        nc.scalar.activation(
                            out=oslice, in_=cslice,
                            func=mybir.ActivationFunctionType.Relu,
                            scale=a2[:, 0:1], bias=c2[:, 0:1])
                    else:
                        nc.vector.tensor_scalar(
                            out=oslice, in0=cslice,
                            scalar1=a2[:, 0:1], scalar2=c2[:, 0:1],
                            op0=mybir.AluOpType.mult,
                            op1=mybir.AluOpType.add)
                        nc.vector.tensor_scalar_max(oslice, oslice, 0.0)
                    dma_eng = nc.sync if n < 2 else nc.gpsimd
                    dma_eng.dma_start(out=out_d[n, :, yb:yb + 28, :],
                                      in_=oslice)

        except _StopBuild:
            pass

    nc.compile()
    return nc


_CACHE = {}


def _get_nc(conv_w, gamma1, beta1, conv2_w, gamma2, beta2):
    import hashlib
    key = hashlib.sha256(b''.join(np.ascontiguousarray(a).tobytes()
                                  for a in (conv_w, gamma1, beta1, conv2_w,
                                            gamma2, beta2))).hexdigest()
    if key not in _CACHE:
        _CACHE[key] = _build(conv_w, gamma1, beta1, conv2_w, gamma2, beta2)
    return _CACHE[key]


def kernel(inp, conv_w, gamma1, beta1, conv2_w, gamma2, beta2):
    inp = np.asarray(inp, np.float32)
    nc = _get_nc(np.asarray(conv_w, np.float32), np.asarray(gamma1, np.float32),
                 np.asarray(beta1, np.float32), np.asarray(conv2_w, np.float32),
                 np.asarray(gamma2, np.float32), np.asarray(beta2, np.float32))
    in_maps = [{"inp": np.ascontiguousarray(inp[i * NB:(i + 1) * NB])}
               for i in range(N_CORES)]
    res = run_bass_kernel_spmd(nc, in_maps, list(range(N_CORES)))
    return np.concatenate([np.asarray(res.results[i]["out"], np.float32)
                           for i in range(N_CORES)], axis=0)
